# revision 1
# baseline (speedup 1.0000x reference)
"""Trainium2 Bass kernel for nn_CumulativeFlattenedLinear (segment_reduce).

Computation: per window of S=64 timesteps, per-timestep C->O linear projection
(weights zero for the first n_discard steps) followed by a causal cumsum within
the window, plus bias.

Strategy (data-parallel over batch, 1 batch element per core):
  - Reformulate per 8-step sub-block u: a triangular-masked "intra" matmul plus
    a "prefix" matmul whose target axis is the later sub-blocks; both share a
    transposed-x stationary and are issued as ONE stacked N=256 fp32r matmul
    writing [intra | pre] contiguously in PSUM (pre region shared per u-pair,
    accumulated in PSUM).
  - x is loaded with partition = 256-element time chunk (1KB contiguous DMA
    runs), shuffled on-chip to (u, c, v') column order (GPSIMD), transposed
    128x128 on the TensorEngine, rounded to fp32r during the batched
    PSUM->SBUF copies (ScalarE).
  - prefix totals summed across the 3 pair-regions + bias (DVE), then one
    strided combine per window writes the (o, t)-ordered output tile, stored
    with 1KB contiguous runs.
"""
import numpy as np

import concourse.bass as bass
import concourse.tile as tile
from concourse import bacc, mybir
from concourse.bass_utils import run_bass_kernel_spmd

F32 = mybir.dt.float32
F32R = mybir.dt.float32r

# problem geometry (asserted against inputs at runtime)
B, C, T, O = 8, 16, 131072, 16
P = 128
CH = 256                 # time-elements per partition per supertile
NST = T // (P * CH)      # 4 supertiles
V = 8                    # sub-block length
NU = 8                   # sub-blocks per window

_cache = {}


def _build_nc(du_count, mm_dtype=F32R):
    """Build the per-core Bass program. du_count = number of active sub-blocks
    (those with any nonzero weight), assumed to be the trailing ones."""
    S = NU * V  # 64
    NW = CH // S  # windows per partition = 4
    DU = du_count
    first_u = NU - DU          # first active sub-block
    fill_s = first_u * V       # s < fill_s -> output = bias

    nc = bacc.Bacc("TRN2", target_bir_lowering=False, debug=False)
    x_d = nc.dram_tensor("x", (C, T), F32, kind="ExternalInput")
    w_d = nc.dram_tensor("w_all", (P, DU * 256), mm_dtype, kind="ExternalInput")
    bpre_d = nc.dram_tensor("bias_pre", (P, P), F32, kind="ExternalInput")
    ident_d = nc.dram_tensor("ident", (P, P), F32, kind="ExternalInput")
    bfill_d = nc.dram_tensor("bias_fill", (P, O * fill_s), F32,
                             kind="ExternalInput")
    y_d = nc.dram_tensor("y", (O, T), F32, kind="ExternalOutput")

    xv = x_d.ap().rearrange("c (st p hs) -> st p c hs", st=NST, p=P, hs=CH)
    yv = y_d.ap().rearrange("o (st p hs) -> st p o hs", st=NST, p=P, hs=CH)

    NB = (DU + 1) // 2  # psum banks per window group

    with tile.TileContext(nc) as tc:
        with (
            tc.tile_pool(name="const", bufs=1) as cp,
            tc.tile_pool(name="io", bufs=2) as io,
            tc.tile_pool(name="mid", bufs=3) as mid,
            tc.tile_pool(name="psW", bufs=2, space="PSUM") as psW,
            tc.tile_pool(name="psT", bufs=2, space="PSUM") as psT,
        ):
            w_all = cp.tile([P, DU * 256], mm_dtype, name="w_all")
            nc.sync.dma_start(w_all[:], w_d.ap())
            bias_pre = cp.tile([P, P], F32, name="bias_pre")
            nc.sync.dma_start(bias_pre[:], bpre_d.ap())
            ident = cp.tile([P, P], F32, name="ident")
            nc.sync.dma_start(ident[:], ident_d.ap())
            bfill = cp.tile([P, O * fill_s], F32, name="bfill")
            nc.sync.dma_start(bfill[:], bfill_d.ap())

            for st in range(NST):
                xin = io.tile([P, C * CH], F32, name="xin", tag="xin")
                nc.sync.dma_start(
                    xin[:].rearrange("p (c hs) -> p c hs", c=C), xv[st]
                )
                out_sb = io.tile([P, O * CH], F32, name="out_sb", tag="out")
                for wdw in range(NW):
                    # ---- shuffle to (du, c, v') ----
                    shuf = mid.tile([P, DU * 128], F32, name="shuf", tag="shuf")
                    src = xin[:].rearrange(
                        "p (c w u v) -> w p u c v", c=C, w=NW, u=NU, v=V
                    )[wdw, :, first_u:NU]
                    nc.vector.tensor_copy(
                        shuf[:].rearrange("p (u c v) -> p u c v", u=DU, c=C, v=V),
                        src,
                    )
                    # ---- transposes (PE) in groups of <=4 per psum bank ----
                    tsb = []
                    du = 0
                    gi = 0
                    while du < DU:
                        n = min(4, DU - du)
                        pt = psT.tile([P, 512], F32, name=f"pt{gi}", tag="pt")
                        for j in range(n):
                            nc.tensor.transpose(
                                pt[:, j * 128:(j + 1) * 128],
                                shuf[:, (du + j) * 128:(du + j + 1) * 128],
                                ident[:],
                                tile_position=(0, 0),
                            )
                        ts = mid.tile([P, n * 128], mm_dtype,
                                      name=f"ts{gi}", tag=f"ts{gi}")
                        nc.scalar.copy(ts[:], pt[:, 0:n * 128])
                        for j in range(n):
                            tsb.append((ts, j))
                        du += n
                        gi += 1
                    # ---- matmuls ----
                    pw = psW.tile([P, NB * 512], F32, name="pw", tag="pw")
                    for du in range(DU):
                        bk = du // 2
                        lo = bk * 512 + (du % 2) * 128
                        ts, j = tsb[du]
                        nc.tensor.matmul(
                            pw[:, lo:lo + 256],
                            ts[:, j * 128:(j + 1) * 128],
                            w_all[:, du * 256:(du + 1) * 256],
                            start=(du % 2 == 0),
                            stop=(du % 2 == 1 or du == DU - 1),
                            skip_group_check=True,
                        )
                    # ---- prefix totals: pre_s = sum of pre regions ----
                    pre_s = mid.tile([P, P], F32, name="pre_s", tag="pre_s")
                    nc.vector.tensor_add(pre_s[:], bias_pre[:], pw[:, 128:256])
                    for bk in range(1, NB):
                        nc.vector.tensor_add(
                            pre_s[:], pre_s[:],
                            pw[:, bk * 512 + 128:bk * 512 + 256],
                        )
                    # ---- combine: out[(o, s)] = intra + pre_bcast ----
                    # out col = o*CH + wdw*S + s,  s = (first_u + du)*V + v
                    out4 = out_sb[:].rearrange(
                        "p (o w u v) -> w p o u v", o=O, w=NW, u=NU, v=V
                    )[wdw, :, :, first_u:NU]
                    # in1: psum intra: col = bk*512 + (du%2)*256 + v*16 + o
                    in1 = pw[:].rearrange(
                        "p (bk half x) -> p bk half x", bk=NB, half=2
                    )[:, :, :, 0:128]
                    in1 = in1.rearrange(
                        "p bk half (v o) -> p o (bk half) v", v=V, o=O
                    )
                    # in2: pre_s col = (first_u + du)*16 + o, step0 over v
                    in2 = pre_s[:].rearrange("p (u o) -> p u o", u=NU)
                    in2 = in2[:, first_u:NU]
                    in2 = in2.transpose([0, 2, 1]).unsqueeze(3)
                    in2 = in2.broadcast_to([P, O, DU, V])
                    nc.vector.tensor_add(out4, in1, in2)
                    # ---- bias fill for s < fill_s (ACT) ----
                    outf = out_sb[:].rearrange(
                        "p (o w s) -> w p o s", o=O, w=NW
                    )[wdw, :, :, 0:fill_s]
                    nc.scalar.copy(
                        outf,
                        bfill[:].rearrange("p (o s) -> p o s", o=O),
                    )
                nc.scalar.dma_start(
                    yv[st], out_sb[:].rearrange("p (o hs) -> p o hs", o=O)
                )
    nc.compile()
    return nc


def _host_constants(weight, bias, n_discard, n_keep, mm_np=np.float32):
    S = n_discard + n_keep
    assert S == NU * V
    w = weight.reshape(O, C, n_keep).transpose(2, 1, 0)  # (n_keep, C, O)
    w_full = np.concatenate(
        [np.zeros((n_discard, C, O), np.float32), w.astype(np.float32)], axis=0
    )  # (S, C, O)
    act = [u for u in range(NU)
           if np.abs(w_full[u * V:(u + 1) * V]).max() > 0]
    # kernel assumes active blocks are trailing & contiguous
    first_u = act[0] if act else NU
    assert act == list(range(first_u, NU))
    DU = len(act)
    rhs = np.zeros((DU, P, 256), np.float32)
    vp_idx = np.arange(V)
    for idx, u in enumerate(act):
        blk = w_full[u * V:(u + 1) * V]  # (V, C, O)
        # Wtri: k=(c,vp) -> n=(v,o)
        tri = np.zeros((C, V, V, O), np.float32)
        for v in range(V):
            tri[:, vp_idx <= v, v, :] = blk.transpose(1, 0, 2)[:, vp_idx <= v]
        Wtri = tri.reshape(C * V, V * O)
        # Wpre: k=(c,vp) -> n=(ut,o)
        pre = np.zeros((C, V, NU, O), np.float32)
        for ut in range(NU):
            if ut > u:
                pre[:, :, ut, :] = blk.transpose(1, 0, 2)
        Wpre = pre.reshape(C * V, NU * O)
        if idx % 2 == 0:
            rhs[idx] = np.concatenate([Wtri, Wpre], axis=1)
        else:
            rhs[idx] = np.concatenate([Wpre, Wtri], axis=1)
    w_all = rhs.transpose(1, 0, 2).reshape(P, DU * 256).astype(mm_np)
    bias32 = bias.astype(np.float32)
    consts = {
        "w_all": np.ascontiguousarray(w_all),
        "bias_pre": np.ascontiguousarray(
            np.tile(bias32, NU)[None, :] * np.ones((P, 1), np.float32)
        ),
        "ident": np.eye(P, dtype=np.float32),
        "bias_fill": np.ascontiguousarray(
            np.tile(bias32[:, None], (1, first_u * V)).reshape(1, -1)
            * np.ones((P, 1), np.float32)
        ),
    }
    return consts, DU


def _run(inputs, trace=False):
    x = np.asarray(inputs["x"], dtype=np.float32)
    weight = np.asarray(inputs["weight"], dtype=np.float32)
    bias = np.asarray(inputs["bias"], dtype=np.float32)
    n_discard = int(inputs["n_discard"])
    n_keep = int(inputs["n_keep"])
    assert x.shape == (B, C, T) and weight.shape == (O, C * n_keep)

    consts, DU = _host_constants(weight, bias, n_discard, n_keep)
    key = ("nc", DU)
    if key not in _cache:
        _cache[key] = _build_nc(DU)
    nc = _cache[key]

    in_maps = []
    for b in range(B):
        m = dict(consts)
        m["x"] = np.ascontiguousarray(x[b])
        in_maps.append(m)
    res = run_bass_kernel_spmd(nc, in_maps, list(range(B)), trace=trace)
    y = np.stack([res.results[b]["y"] for b in range(B)], axis=0)
    return y, res


def kernel(**inputs):
    y, _ = _run(inputs, trace=False)
    return y



# revision 2
# speedup vs baseline: 1.1646x; 1.1646x over previous
"""Trainium2 Bass kernel for nn_CumulativeFlattenedLinear (segment_reduce).

Computation: per window of S=64 timesteps, per-timestep C->O linear projection
(weights zero for the first n_discard steps) followed by a causal cumsum within
the window, plus bias.

Strategy (data-parallel over batch, 1 batch element per core):
  - Per 8-step sub-block u: a triangular-masked "intra" matmul plus a "prefix"
    matmul targeting later sub-blocks; both issued as ONE stacked N=256 bf16
    matmul writing [intra | pre] contiguously in PSUM (pre region shared per
    u-pair, accumulated in PSUM).
  - x is loaded with partition = 256-element time chunk (1KB contiguous DMA
    runs, split in two half-tile DMAs so compute starts early), shuffled+cast
    to bf16 in (u, c, v') column order on the Scalar engine, transposed
    128x128 on the TensorEngine (bf16: 1 cyc/row), copied PSUM->SBUF (Scalar).
  - prefix totals summed across the 3 pair-regions + bias (DVE), then one
    strided combine per window writes the (o, t)-ordered output tile; the
    bias-only head region (s < n_discard) is filled once per supertile (DVE).
  - Output stored in two half-tile DMAs per supertile (Scalar queue) so the
    store stream overlaps the input stream; constants load on the GpSimd
    queue so the first x tile is the first packet on the input queue.
"""
import numpy as np
import ml_dtypes

import concourse.bass as bass
import concourse.tile as tile
from concourse import bacc, mybir
from concourse.bass_utils import run_bass_kernel_spmd

F32 = mybir.dt.float32
BF16 = mybir.dt.bfloat16

# problem geometry (asserted against inputs at runtime)
B, C, T, O = 8, 16, 131072, 16
P = 128
CH = 256                 # time-elements per partition per supertile
NST = T // (P * CH)      # 4 supertiles
V = 8                    # sub-block length
NU = 8                   # sub-blocks per window

_cache = {}


def _build_nc(du_count):
    """Build the per-core Bass program. du_count = number of active sub-blocks
    (those with any nonzero weight), assumed to be the trailing ones."""
    S = NU * V  # 64
    NW = CH // S  # windows per partition = 4
    DU = du_count
    first_u = NU - DU          # first active sub-block
    fill_s = first_u * V       # s < fill_s -> output = bias

    nc = bacc.Bacc("TRN2", target_bir_lowering=False, debug=False)
    x_d = nc.dram_tensor("x", (C, T), F32, kind="ExternalInput")
    w_d = nc.dram_tensor("w_all", (P, DU * 256), BF16, kind="ExternalInput")
    bpre_d = nc.dram_tensor("bias_pre", (P, P), F32, kind="ExternalInput")
    ident_d = nc.dram_tensor("ident", (P, P), BF16, kind="ExternalInput")
    y_d = nc.dram_tensor("y", (O, T), F32, kind="ExternalOutput")

    xv = x_d.ap().rearrange("c (st p hs) -> st p c hs", st=NST, p=P, hs=CH)
    yv = y_d.ap().rearrange("o (st p hs) -> st p o hs", st=NST, p=P, hs=CH)

    NB = (DU + 1) // 2  # psum banks per window group
    H = CH // 2         # half-tile split for load/store DMAs

    with tile.TileContext(nc) as tc:
        with (
            tc.tile_pool(name="const", bufs=1) as cp,
            tc.tile_pool(name="io", bufs=3) as io,
            tc.tile_pool(name="mid", bufs=3) as mid,
            tc.tile_pool(name="psW", bufs=2, space="PSUM") as psW,
            tc.tile_pool(name="psT", bufs=2, space="PSUM") as psT,
        ):
            # constants on the GpSimd DMA queue: the Sync queue's first
            # descriptors are then the first x half-tile.
            w_all = cp.tile([P, DU * 256], BF16, name="w_all")
            nc.gpsimd.dma_start(w_all[:], w_d.ap())
            bias_pre = cp.tile([P, P], F32, name="bias_pre")
            nc.gpsimd.dma_start(bias_pre[:], bpre_d.ap())
            ident = cp.tile([P, P], BF16, name="ident")
            nc.gpsimd.dma_start(ident[:], ident_d.ap())

            for st in range(NST):
                xin = io.tile([P, C * CH], F32, name="xin", tag="xin")
                xin_v = xin[:].rearrange("p (c hs) -> p c hs", c=C)
                nc.sync.dma_start(xin_v[:, :, 0:H], xv[st][:, :, 0:H])
                nc.sync.dma_start(xin_v[:, :, H:CH], xv[st][:, :, H:CH])
                out_sb = io.tile([P, O * CH], F32, name="out_sb", tag="out")
                # ---- bias fill for s < fill_s, all windows of this st (DVE)
                if fill_s:
                    outf = out_sb[:].rearrange(
                        "p (o w u v) -> p o w u v", o=O, w=NW, u=NU, v=V
                    )[:, :, :, 0:first_u]
                    bsrc = (
                        bias_pre[:, 0:O]
                        .unsqueeze(2).unsqueeze(3).unsqueeze(4)
                        .broadcast_to([P, O, NW, first_u, V])
                    )
                    nc.vector.tensor_copy(outf, bsrc)
                for wdw in range(NW):
                    # ---- shuffle+cast to (du, c, v) bf16 (Scalar) ----
                    shuf = mid.tile([P, DU * 128], BF16, name="shuf", tag="shuf")
                    src = xin[:].rearrange(
                        "p (c w u v) -> w p u c v", c=C, w=NW, u=NU, v=V
                    )[wdw, :, first_u:NU]
                    nc.scalar.copy(
                        shuf[:].rearrange("p (u c v) -> p u c v", u=DU, c=C, v=V),
                        src,
                    )
                    # ---- transposes (PE) in groups of <=4 per psum tile ----
                    tsb = []
                    du = 0
                    gi = 0
                    while du < DU:
                        n = min(4, DU - du)
                        pt = psT.tile([P, 512], BF16, name=f"pt{gi}", tag="pt")
                        for j in range(n):
                            nc.tensor.transpose(
                                pt[:, j * 128:(j + 1) * 128],
                                shuf[:, (du + j) * 128:(du + j + 1) * 128],
                                ident[:],
                                tile_position=(0, 0),
                            )
                        ts = mid.tile([P, n * 128], BF16,
                                      name=f"ts{gi}", tag=f"ts{gi}")
                        nc.scalar.copy(ts[:], pt[:, 0:n * 128])
                        for j in range(n):
                            tsb.append((ts, j))
                        du += n
                        gi += 1
                    # ---- matmuls ----
                    pw = psW.tile([P, NB * 512], F32, name="pw", tag="pw")
                    for du in range(DU):
                        bk = du // 2
                        lo = bk * 512 + (du % 2) * 128
                        ts, j = tsb[du]
                        nc.tensor.matmul(
                            pw[:, lo:lo + 256],
                            ts[:, j * 128:(j + 1) * 128],
                            w_all[:, du * 256:(du + 1) * 256],
                            start=(du % 2 == 0),
                            stop=(du % 2 == 1 or du == DU - 1),
                            skip_group_check=True,
                        )
                    # ---- prefix totals: pre_s = bias + sum of pre regions ----
                    pre_s = mid.tile([P, P], F32, name="pre_s", tag="pre_s")
                    nc.vector.tensor_add(pre_s[:], bias_pre[:], pw[:, 128:256])
                    for bk in range(1, NB):
                        nc.vector.tensor_add(
                            pre_s[:], pre_s[:],
                            pw[:, bk * 512 + 128:bk * 512 + 256],
                        )
                    # ---- combine: out[(o, s)] = intra + pre_bcast ----
                    out4 = out_sb[:].rearrange(
                        "p (o w u v) -> w p o u v", o=O, w=NW, u=NU, v=V
                    )[wdw, :, :, first_u:NU]
                    in1 = pw[:].rearrange(
                        "p (bk half x) -> p bk half x", bk=NB, half=2
                    )[:, :, :, 0:128]
                    in1 = in1.rearrange(
                        "p bk half (v o) -> p o (bk half) v", v=V, o=O
                    )
                    in2 = pre_s[:].rearrange("p (u o) -> p u o", u=NU)
                    in2 = in2[:, first_u:NU]
                    in2 = in2.transpose([0, 2, 1]).unsqueeze(3)
                    in2 = in2.broadcast_to([P, O, DU, V])
                    nc.vector.tensor_add(out4, in1, in2)
                    # ---- half-tile stores overlap with remaining compute ----
                    if wdw == NW // 2 - 1:
                        nc.scalar.dma_start(
                            yv[st][:, :, 0:H],
                            out_sb[:].rearrange(
                                "p (o hs) -> p o hs", o=O)[:, :, 0:H],
                        )
                    elif wdw == NW - 1:
                        nc.scalar.dma_start(
                            yv[st][:, :, H:CH],
                            out_sb[:].rearrange(
                                "p (o hs) -> p o hs", o=O)[:, :, H:CH],
                        )
    nc.compile()
    return nc


def _host_constants(weight, bias, n_discard, n_keep):
    S = n_discard + n_keep
    assert S == NU * V
    w = weight.reshape(O, C, n_keep).transpose(2, 1, 0)  # (n_keep, C, O)
    w_full = np.concatenate(
        [np.zeros((n_discard, C, O), np.float32), w.astype(np.float32)], axis=0
    )  # (S, C, O)
    act = [u for u in range(NU)
           if np.abs(w_full[u * V:(u + 1) * V]).max() > 0]
    # kernel assumes active blocks are trailing & contiguous
    first_u = act[0] if act else NU
    assert act == list(range(first_u, NU))
    DU = len(act)
    rhs = np.zeros((DU, P, 256), np.float32)
    vp_idx = np.arange(V)
    for idx, u in enumerate(act):
        blk = w_full[u * V:(u + 1) * V]  # (V, C, O)
        # Wtri: k=(c,vp) -> n=(v,o)
        tri = np.zeros((C, V, V, O), np.float32)
        for v in range(V):
            tri[:, vp_idx <= v, v, :] = blk.transpose(1, 0, 2)[:, vp_idx <= v]
        Wtri = tri.reshape(C * V, V * O)
        # Wpre: k=(c,vp) -> n=(ut,o)
        pre = np.zeros((C, V, NU, O), np.float32)
        for ut in range(NU):
            if ut > u:
                pre[:, :, ut, :] = blk.transpose(1, 0, 2)
        Wpre = pre.reshape(C * V, NU * O)
        if idx % 2 == 0:
            rhs[idx] = np.concatenate([Wtri, Wpre], axis=1)
        else:
            rhs[idx] = np.concatenate([Wpre, Wtri], axis=1)
    w_all = rhs.transpose(1, 0, 2).reshape(P, DU * 256)
    bias32 = bias.astype(np.float32)
    consts = {
        "w_all": np.ascontiguousarray(w_all).astype(ml_dtypes.bfloat16),
        "bias_pre": np.ascontiguousarray(
            np.tile(bias32, NU)[None, :] * np.ones((P, 1), np.float32)
        ),
        "ident": np.eye(P, dtype=np.float32).astype(ml_dtypes.bfloat16),
    }
    return consts, DU


def _run(inputs, trace=False):
    x = np.asarray(inputs["x"], dtype=np.float32)
    weight = np.asarray(inputs["weight"], dtype=np.float32)
    bias = np.asarray(inputs["bias"], dtype=np.float32)
    n_discard = int(inputs["n_discard"])
    n_keep = int(inputs["n_keep"])
    assert x.shape == (B, C, T) and weight.shape == (O, C * n_keep)

    consts, DU = _host_constants(weight, bias, n_discard, n_keep)
    key = ("nc", DU)
    if key not in _cache:
        _cache[key] = _build_nc(DU)
    nc = _cache[key]

    in_maps = []
    for b in range(B):
        m = dict(consts)
        m["x"] = np.ascontiguousarray(x[b])
        in_maps.append(m)
    res = run_bass_kernel_spmd(nc, in_maps, list(range(B)), trace=trace)
    y = np.stack([res.results[b]["y"] for b in range(B)], axis=0)
    return y, res


def kernel(**inputs):
    y, _ = _run(inputs, trace=False)
    return y


# revision 5
# speedup vs baseline: 1.3706x; 1.1769x over previous
"""Trainium2 Bass kernel for nn_CumulativeFlattenedLinear (segment_reduce).

Computation: per window of S=64 timesteps, per-timestep C->O linear projection
(weights zero for the first n_discard steps) followed by a causal cumsum within
the window, plus bias.

Strategy (data-parallel over batch, 1 batch element per core):
  - Per 8-step sub-block u: a triangular-masked "intra" matmul plus a "prefix"
    matmul targeting later sub-blocks; both issued as ONE stacked N=256 bf16
    matmul writing [intra | pre] contiguously in PSUM (pre region shared per
    u-pair, accumulated in PSUM).
  - x is loaded with partition = 256-element time chunk (1KB contiguous DMA
    runs, split in two half-tile DMAs so compute starts early), shuffled+cast
    to bf16 in (u, c, v') column order on the Scalar engine, transposed
    128x128 on the TensorEngine (bf16: 1 cyc/row), copied PSUM->SBUF (Scalar).
  - prefix totals summed across the 3 pair-regions + bias (DVE), then one
    strided combine per window writes the (o, t)-ordered output tile; the
    bias-only head region (s < n_discard) is filled once per supertile (DVE).
  - Output stored in two half-tile DMAs per supertile (Scalar queue) so the
    store stream overlaps the input stream; constants load on the GpSimd
    queue so the first x tile is the first packet on the input queue.
"""
import numpy as np
import ml_dtypes

import concourse.bass as bass
import concourse.tile as tile
from concourse import bacc, mybir
from concourse.bass_utils import run_bass_kernel_spmd

F32 = mybir.dt.float32
BF16 = mybir.dt.bfloat16

# problem geometry (asserted against inputs at runtime)
B, C, T, O = 8, 16, 131072, 16
P = 128
CH = 256                 # time-elements per partition per supertile
NST = T // (P * CH)      # 4 supertiles
V = 8                    # sub-block length
NU = 8                   # sub-blocks per window

_cache = {}


def _build_nc(du_count):
    """Build the per-core Bass program. du_count = number of active sub-blocks
    (those with any nonzero weight), assumed to be the trailing ones."""
    S = NU * V  # 64
    NW = CH // S  # windows per partition = 4
    DU = du_count
    first_u = NU - DU          # first active sub-block
    fill_s = first_u * V       # s < fill_s -> output = bias

    nc = bacc.Bacc("TRN2", target_bir_lowering=False, debug=False)
    x_d = nc.dram_tensor("x", (C, T), F32, kind="ExternalInput")
    w_d = nc.dram_tensor("w_all", (P, DU * 256), BF16, kind="ExternalInput")
    bpre_d = nc.dram_tensor("bias_pre", (P, P), F32, kind="ExternalInput")
    ident_d = nc.dram_tensor("ident", (P, P), BF16, kind="ExternalInput")
    y_d = nc.dram_tensor("y", (O, T), F32, kind="ExternalOutput")

    xv = x_d.ap().rearrange("c (st p hs) -> st p c hs", st=NST, p=P, hs=CH)
    yv = y_d.ap().rearrange("o (st p hs) -> st p o hs", st=NST, p=P, hs=CH)

    NB = (DU + 1) // 2  # psum banks per window group
    H = CH // 2         # half-tile split for load/store DMAs

    with tile.TileContext(nc) as tc:
        with (
            tc.tile_pool(name="const", bufs=1) as cp,
            tc.tile_pool(name="io", bufs=3) as io,
            tc.tile_pool(name="mid", bufs=3) as mid,
            tc.tile_pool(name="psW", bufs=2, space="PSUM") as psW,
            tc.tile_pool(name="psT", bufs=2, space="PSUM") as psT,
        ):
            # constants on the GpSimd DMA queue: the Sync queue's first
            # descriptors are then the first x half-tile.
            w_all = cp.tile([P, DU * 256], BF16, name="w_all")
            nc.gpsimd.dma_start(w_all[:], w_d.ap())
            bias_pre = cp.tile([P, P], F32, name="bias_pre")
            nc.gpsimd.dma_start(bias_pre[:], bpre_d.ap())
            ident = cp.tile([P, P], BF16, name="ident")
            nc.gpsimd.dma_start(ident[:], ident_d.ap())

            for st in range(NST):
                xin = io.tile([P, C * CH], F32, name="xin", tag="xin")
                nc.sync.dma_start(
                    xin[:].rearrange("p (c hs) -> p c hs", c=C), xv[st]
                )
                out_sb = io.tile([P, O * CH], F32, name="out_sb", tag="out")
                # ---- bias fill for s < fill_s, all windows of this st ----
                if fill_s:
                    outf = out_sb[:].rearrange(
                        "p (o w u v) -> p o w u v", o=O, w=NW, u=NU, v=V
                    )[:, :, :, 0:first_u]
                    bsrc = (
                        bias_pre[:, 0:O]
                        .unsqueeze(2).unsqueeze(3).unsqueeze(4)
                        .broadcast_to([P, O, NW, first_u, V])
                    )
                    nc.gpsimd.tensor_copy(outf, bsrc)
                for wdw in range(NW):
                    # ---- shuffle+cast to (du, c, v) bf16 (Scalar) ----
                    shuf = mid.tile([P, DU * 128], BF16, name="shuf", tag="shuf")
                    src = xin[:].rearrange(
                        "p (c w u v) -> w p u c v", c=C, w=NW, u=NU, v=V
                    )[wdw, :, first_u:NU]
                    nc.scalar.copy(
                        shuf[:].rearrange("p (u c v) -> p u c v", u=DU, c=C, v=V),
                        src,
                    )
                    # ---- transposes (PE): all DU into one bf16 psum bank ----
                    pt = psT.tile([P, DU * 128], BF16, name="pt", tag="pt")
                    for j in range(DU):
                        nc.tensor.transpose(
                            pt[:, j * 128:(j + 1) * 128],
                            shuf[:, j * 128:(j + 1) * 128],
                            ident[:],
                            tile_position=(0, 0),
                        )
                    ts = mid.tile([P, DU * 128], BF16, name="ts", tag="ts")
                    nc.scalar.copy(ts[:], pt[:])
                    # ---- matmuls ----
                    pw = psW.tile([P, NB * 512], F32, name="pw", tag="pw")
                    for du in range(DU):
                        bk = du // 2
                        lo = bk * 512 + (du % 2) * 128
                        nc.tensor.matmul(
                            pw[:, lo:lo + 256],
                            ts[:, du * 128:(du + 1) * 128],
                            w_all[:, du * 256:(du + 1) * 256],
                            start=(du % 2 == 0),
                            stop=(du % 2 == 1 or du == DU - 1),
                            skip_group_check=True,
                        )
                    # ---- prefix totals: pre_s = bias + sum of pre regions ----
                    pre_s = mid.tile([P, P], F32, name="pre_s", tag="pre_s")
                    nc.vector.tensor_add(pre_s[:], bias_pre[:], pw[:, 128:256])
                    for bk in range(1, NB):
                        nc.vector.tensor_add(
                            pre_s[:], pre_s[:],
                            pw[:, bk * 512 + 128:bk * 512 + 256],
                        )
                    # ---- combine: out[(o, s)] = intra + pre_bcast ----
                    out4 = out_sb[:].rearrange(
                        "p (o w u v) -> w p o u v", o=O, w=NW, u=NU, v=V
                    )[wdw, :, :, first_u:NU]
                    in1 = pw[:].rearrange(
                        "p (bk half x) -> p bk half x", bk=NB, half=2
                    )[:, :, :, 0:128]
                    in1 = in1.rearrange(
                        "p bk half (v o) -> p o (bk half) v", v=V, o=O
                    )
                    in2 = pre_s[:].rearrange("p (u o) -> p u o", u=NU)
                    in2 = in2[:, first_u:NU]
                    in2 = in2.transpose([0, 2, 1]).unsqueeze(3)
                    in2 = in2.broadcast_to([P, O, DU, V])
                    nc.vector.tensor_add(out4, in1, in2)
                nc.scalar.dma_start(
                    yv[st], out_sb[:].rearrange("p (o hs) -> p o hs", o=O)
                )
    nc.compile()
    return nc


def _host_constants(weight, bias, n_discard, n_keep):
    S = n_discard + n_keep
    assert S == NU * V
    w = weight.reshape(O, C, n_keep).transpose(2, 1, 0)  # (n_keep, C, O)
    w_full = np.concatenate(
        [np.zeros((n_discard, C, O), np.float32), w.astype(np.float32)], axis=0
    )  # (S, C, O)
    act = [u for u in range(NU)
           if np.abs(w_full[u * V:(u + 1) * V]).max() > 0]
    # kernel assumes active blocks are trailing & contiguous
    first_u = act[0] if act else NU
    assert act == list(range(first_u, NU))
    DU = len(act)
    rhs = np.zeros((DU, P, 256), np.float32)
    vp_idx = np.arange(V)
    for idx, u in enumerate(act):
        blk = w_full[u * V:(u + 1) * V]  # (V, C, O)
        # Wtri: k=(c,vp) -> n=(v,o)
        tri = np.zeros((C, V, V, O), np.float32)
        for v in range(V):
            tri[:, vp_idx <= v, v, :] = blk.transpose(1, 0, 2)[:, vp_idx <= v]
        Wtri = tri.reshape(C * V, V * O)
        # Wpre: k=(c,vp) -> n=(ut,o)
        pre = np.zeros((C, V, NU, O), np.float32)
        for ut in range(NU):
            if ut > u:
                pre[:, :, ut, :] = blk.transpose(1, 0, 2)
        Wpre = pre.reshape(C * V, NU * O)
        if idx % 2 == 0:
            rhs[idx] = np.concatenate([Wtri, Wpre], axis=1)
        else:
            rhs[idx] = np.concatenate([Wpre, Wtri], axis=1)
    w_all = rhs.transpose(1, 0, 2).reshape(P, DU * 256)
    bias32 = bias.astype(np.float32)
    consts = {
        "w_all": np.ascontiguousarray(w_all).astype(ml_dtypes.bfloat16),
        "bias_pre": np.ascontiguousarray(
            np.tile(bias32, NU)[None, :] * np.ones((P, 1), np.float32)
        ),
        "ident": np.eye(P, dtype=np.float32).astype(ml_dtypes.bfloat16),
    }
    return consts, DU


def _run(inputs, trace=False):
    x = np.asarray(inputs["x"], dtype=np.float32)
    weight = np.asarray(inputs["weight"], dtype=np.float32)
    bias = np.asarray(inputs["bias"], dtype=np.float32)
    n_discard = int(inputs["n_discard"])
    n_keep = int(inputs["n_keep"])
    assert x.shape == (B, C, T) and weight.shape == (O, C * n_keep)

    consts, DU = _host_constants(weight, bias, n_discard, n_keep)
    key = ("nc", DU)
    if key not in _cache:
        _cache[key] = _build_nc(DU)
    nc = _cache[key]

    in_maps = []
    for b in range(B):
        m = dict(consts)
        m["x"] = np.ascontiguousarray(x[b])
        in_maps.append(m)
    res = run_bass_kernel_spmd(nc, in_maps, list(range(B)), trace=trace)
    y = np.stack([res.results[b]["y"] for b in range(B)], axis=0)
    return y, res


def kernel(**inputs):
    y, _ = _run(inputs, trace=False)
    return y
